# revision 5
# baseline (speedup 1.0000x reference)
"""DetectionLoss Trainium2 kernel.

Strategy (data-parallel over batch, per sharding hint):
- Shard B=32 across 8 cores (4 images each).
- Host-side prep per core: transpose feature shards to channel-last and
  concatenate all 3 pyramid levels into one [33600, 144] DRAM tensor, so each
  target's 144 channel values are one contiguous row; precompute gather row
  indices, one-hot/mask and DFL weight tensors from the (tiny) target tensors.
- Device: a single indirect DMA gathers the 144-float feature row of all
  768 = 4 img * 64 tgt * 3 layer targets; focal cls loss and DFL box loss are
  computed on-chip (exp/ln on Act engine, elementwise/reduce on DVE,
  partition-sum via PE matmul against a ones vector).
- Host: sum the 8 per-core (cls, box) partials -> (total, cls, box).

The full feature maps are shipped to device DRAM but only the ~440KB/core the
loss actually references is ever read by the kernel, so HW time sits far under
the streaming-memory roofline.

All SBUF tensors participating in ops against strided channel slices (cls
[64:144] / dist [0:64]) are allocated [128, 6, 144] and sliced identically, so
every instruction's operands lower to the same access-pattern shape.
"""

import sys
from contextlib import ExitStack

import numpy as np

for _p in ("/opt/trn_rl_repo", "/root/.axon_site/_ro/trn_rl_repo"):
    if _p not in sys.path:
        sys.path.append(_p)

N_CLASSES = 80
N_BINS = 16
ND = 4 * N_BINS             # 64 dist channels
B, T = 32, 64
M = 8                       # cores
BL = B // M                 # images per core
C = N_CLASSES + ND          # 144
HWS = [(80, 80), (40, 40), (20, 20)]
ROWS = 3 * BL * T           # 768 gathered rows per core
NBLK = ROWS // 128          # 6
ROWS_PER_LAYER = BL * T     # 256
N_FT = BL * sum(h * w for h, w in HWS)  # 33600 rows in the concat feature table
LAYER_BASE = [0, BL * 6400, BL * 6400 + BL * 1600]

_PROG = None


def _build_program():
    import concourse.bass as bass
    import concourse.tile as tile
    from concourse import bacc, mybir

    f32 = mybir.dt.float32
    i32 = mybir.dt.int32
    Act = mybir.ActivationFunctionType
    Alu = mybir.AluOpType
    AxX = mybir.AxisListType.X
    AxXY = mybir.AxisListType.XY

    nc = bacc.Bacc("TRN2", debug=False, num_devices=M)

    ft_d = nc.dram_tensor("ft", [N_FT, C], f32, kind="ExternalInput").ap()
    idx_d = nc.dram_tensor("idx", [128, NBLK], i32, kind="ExternalInput").ap()
    aux_d = nc.dram_tensor("aux", [128, NBLK, C], f32, kind="ExternalInput").ap()
    msk_d = nc.dram_tensor("msk", [128, NBLK], f32, kind="ExternalInput").ap()
    ws_d = nc.dram_tensor("ws", [128, NBLK, 4], f32, kind="ExternalInput").ap()
    out_d = nc.dram_tensor("out", [1, 2], f32, kind="ExternalOutput").ap()

    with tile.TileContext(nc) as tc, ExitStack() as ctx:
        sb = ctx.enter_context(tc.tile_pool(name="sb", bufs=1))
        ps = ctx.enter_context(tc.tile_pool(name="ps", bufs=1, space="PSUM"))

        idx = sb.tile([128, NBLK], i32)
        aux = sb.tile([128, NBLK, C], f32)
        msk = sb.tile([128, NBLK], f32)
        ws = sb.tile([128, NBLK, 4], f32)
        nc.sync.dma_start(out=idx[:], in_=idx_d)
        nc.sync.dma_start(out=aux[:], in_=aux_d)
        nc.sync.dma_start(out=msk[:], in_=msk_d)
        nc.sync.dma_start(out=ws[:], in_=ws_d)

        # One indirect DMA gathers every target's 144-float feature row.
        G = sb.tile([128, NBLK, C], f32)
        nc.gpsimd.indirect_dma_start(
            out=G[:],
            out_offset=None,
            in_=ft_d,
            in_offset=bass.IndirectOffsetOnAxis(ap=idx[:, :], axis=0),
        )

        X = G[:, :, ND:]      # [128, 6, 80] class logits
        D = G[:, :, :ND]      # [128, 6, 64] dist logits
        OH = aux[:, :, ND:]   # one-hot(tgt_cls)
        WD = aux[:, :, :ND]   # DFL lo/hi bin weights

        EB = sb.tile([128, NBLK, C], f32)   # exp(G)
        TB = sb.tile([128, NBLK, C], f32)   # G * aux
        S = sb.tile([128, NBLK], f32)
        L = sb.tile([128, NBLK], f32)
        XS = sb.tile([128, NBLK], f32)
        CE = sb.tile([128, NBLK], f32)
        PT = sb.tile([128, NBLK], f32)
        Q2 = sb.tile([128, NBLK], f32)
        F = sb.tile([128, NBLK], f32)
        FM = sb.tile([128, NBLK], f32)
        P2 = sb.tile([128, 2], f32)

        # ---- focal classification loss ----
        nc.scalar.activation(out=EB[:, :, ND:], in_=X, func=Act.Exp)
        nc.vector.tensor_reduce(out=S[:], in_=EB[:, :, ND:], axis=AxX, op=Alu.add)
        nc.scalar.activation(out=L[:], in_=S[:], func=Act.Ln)
        nc.vector.tensor_tensor(out=TB[:, :, ND:], in0=X, in1=OH, op=Alu.mult)
        nc.vector.tensor_reduce(out=XS[:], in_=TB[:, :, ND:], axis=AxX, op=Alu.add)
        nc.vector.tensor_tensor(out=CE[:], in0=L[:], in1=XS[:], op=Alu.subtract)
        nc.scalar.activation(out=PT[:], in_=CE[:], func=Act.Exp, scale=-1.0)
        nc.scalar.activation(out=Q2[:], in_=PT[:], func=Act.Square, scale=-1.0, bias=1.0)
        nc.vector.tensor_tensor(out=F[:], in0=Q2[:], in1=CE[:], op=Alu.mult)
        nc.vector.tensor_tensor(out=FM[:], in0=F[:], in1=msk[:], op=Alu.mult)
        nc.vector.tensor_reduce(out=P2[:, 0:1], in_=FM[:], axis=AxX, op=Alu.add)

        # ---- DFL box loss ----
        # dl = -(lps[lo]*wl + lps[hi]*wr), lps = D - log(sum(exp(D))) per
        # 16-bin group  =>  box = sum(ws * LD) - sum(WD * D) with host-baked
        # sparse weights (ws[.,s] = wl+wr on the selected row/side, WD holds
        # wl/wr at the lo/hi bins of selected rows).
        SD = sb.tile([128, NBLK, 4], f32)
        LD = sb.tile([128, NBLK, 4], f32)
        T1 = sb.tile([128, NBLK, 4], f32)
        Acc1 = sb.tile([128, 1], f32)
        Acc2 = sb.tile([128, 1], f32)

        nc.scalar.activation(out=EB[:, :, :ND], in_=D, func=Act.Exp)
        nc.vector.tensor_reduce(
            out=SD[:],
            in_=EB[:, :, :ND].rearrange("p r (s n) -> p r s n", n=N_BINS),
            axis=AxX,
            op=Alu.add,
            opt_output=False,
        )
        nc.scalar.activation(out=LD[:], in_=SD[:], func=Act.Ln)
        nc.vector.tensor_tensor(out=T1[:], in0=LD[:], in1=ws[:], op=Alu.mult)
        nc.vector.tensor_reduce(out=Acc1[:], in_=T1[:], axis=AxXY, op=Alu.add)
        nc.vector.tensor_tensor(out=TB[:, :, :ND], in0=D, in1=WD, op=Alu.mult)
        nc.vector.tensor_reduce(out=Acc2[:], in_=TB[:, :, :ND], axis=AxXY, op=Alu.add)
        nc.vector.tensor_tensor(out=P2[:, 1:2], in0=Acc1[:], in1=Acc2[:], op=Alu.subtract)

        # ---- partition-dim sum via PE: ones[128,1].T @ P2[128,2] -> [1,2] ----
        ONES = sb.tile([128, 1], f32)
        nc.vector.memset(ONES[:], 1.0)
        PS = ps.tile([1, 2], f32)
        nc.tensor.matmul(out=PS[:], lhsT=ONES[:], rhs=P2[:], start=True, stop=True)
        O = sb.tile([1, 2], f32)
        nc.vector.tensor_copy(out=O[:], in_=PS[:])
        nc.sync.dma_start(out=out_d, in_=O[:])

    nc.compile()
    return nc


def _host_prep(feat0, feat1, feat2, tgt_box, tgt_cls, tgt_layer):
    """Build the 8 per-core input maps."""
    f32 = np.float32
    feats = [feat0, feat1, feat2]
    cx, cy = tgt_box[..., 0], tgt_box[..., 1]
    wv, hv = tgt_box[..., 2], tgt_box[..., 3]

    # Per-layer integer grid positions (bit-exact with the f32 reference math).
    fx, fy = {}, {}
    for li, (H, W) in enumerate(HWS):
        fx[li] = np.clip((cx * f32(W)).astype(np.int32), 0, W - 1)  # [B,T]
        fy[li] = np.clip((cy * f32(H)).astype(np.int32), 0, H - 1)

    # Per-layer DFL quantities (the reference's "last matching target" bug).
    tidx = np.arange(T)
    bv = np.arange(B)
    dfl = {}
    for li, (H, W) in enumerate(HWS):
        mask_l = tgt_layer == li
        last = np.max(np.where(mask_l, tidx[None, :], -1), axis=1)  # [B]
        has = last >= 0
        last_c = np.maximum(last, 0)
        lw = np.maximum(wv[bv, last_c], f32(0.0)) * f32(0.5)
        lh = np.maximum(hv[bv, last_c], f32(0.0)) * f32(0.5)
        gt = np.stack([lw * f32(W), lh * f32(H), lw * f32(W), lh * f32(H)], axis=1)
        tq = np.clip(gt, f32(0.0), f32(N_BINS - 1 - 1e-6))
        lo = np.floor(tq)
        wl = (lo + f32(1.0)) - tq
        wr = tq - lo
        lo_i = lo.astype(np.int32)
        hi_i = np.minimum(lo_i + 1, N_BINS - 1)
        dfl[li] = (last_c, has, wl, wr, lo_i, hi_i)

    blv = np.arange(BL)
    in_maps = []
    for m in range(M):
        b0 = m * BL
        sl = slice(b0, b0 + BL)
        im = {}

        ft = np.empty((N_FT, C), f32)
        for li, (H, W) in enumerate(HWS):
            base = LAYER_BASE[li]
            src = feats[li][sl].reshape(BL, C, H * W).transpose(0, 2, 1)
            ft[base : base + BL * H * W] = src.reshape(BL * H * W, C)
        im["ft"] = ft

        idx = np.zeros((128, NBLK), np.int32)
        aux = np.zeros((128, NBLK, C), f32)
        msk = np.zeros((128, NBLK), f32)
        ws = np.zeros((128, NBLK, 4), f32)

        rr_bt = blv[:, None] * T + tidx[None, :]  # [BL, T] row-within-layer
        for li, (H, W) in enumerate(HWS):
            rr = li * ROWS_PER_LAYER + rr_bt
            p, blk = rr % 128, rr // 128
            idx[p, blk] = (
                LAYER_BASE[li] + blv[:, None] * (H * W) + fy[li][sl] * W + fx[li][sl]
            ).astype(np.int32)
            aux[p, blk, ND + tgt_cls[sl]] = f32(1.0)
            msk[p, blk] = (tgt_layer[sl] == li).astype(f32)

            last_c, has, wl, wr, lo_i, hi_i = dfl[li]
            for bl in range(BL):
                b = b0 + bl
                if not has[b]:
                    continue
                r = li * ROWS_PER_LAYER + bl * T + last_c[b]
                p1, blk1 = r % 128, r // 128
                for s in range(4):
                    aux[p1, blk1, s * N_BINS + lo_i[b, s]] = wl[b, s]
                    aux[p1, blk1, s * N_BINS + hi_i[b, s]] = wr[b, s]
                    ws[p1, blk1, s] = wl[b, s] + wr[b, s]

        im["idx"] = idx
        im["aux"] = aux
        im["msk"] = msk
        im["ws"] = ws
        in_maps.append(im)
    return in_maps


def kernel(feat0, feat1, feat2, tgt_box, tgt_cls, tgt_layer):
    global _PROG
    from concourse.bass_utils import run_bass_kernel_spmd

    in_maps = _host_prep(feat0, feat1, feat2, tgt_box, tgt_cls, tgt_layer)
    if _PROG is None:
        _PROG = _build_program()
    res = run_bass_kernel_spmd(_PROG, in_maps, list(range(M))).results
    parts = np.stack([res[i]["out"][0] for i in range(M)])  # [M, 2]
    cls_tot = parts[:, 0].sum(dtype=np.float32)
    box_tot = parts[:, 1].sum(dtype=np.float32)
    total = np.float32(cls_tot + box_tot)
    return (total, np.float32(cls_tot), np.float32(box_tot))


# revision 8
# speedup vs baseline: 348.5918x; 348.5918x over previous
"""DetectionLoss Trainium2 kernel.

Strategy (data-parallel over batch, per sharding hint):
- Shard B=32 across 8 cores (4 images each).
- Host-side prep per core: transpose feature shards to channel-last and
  concatenate all 3 pyramid levels into one [33600, 144] DRAM tensor, so each
  target's 144 channel values are one contiguous row; precompute gather row
  indices, one-hot/mask and DFL weight tensors from the (tiny) target tensors.
- Device: a single indirect DMA gathers the 144-float feature row of all
  768 = 4 img * 64 tgt * 3 layer targets; focal cls loss and DFL box loss are
  computed on-chip (exp/ln on Act engine, elementwise/reduce on DVE,
  partition-sum via PE matmul against a ones vector).
- Host: sum the 8 per-core (cls, box) partials -> (total, cls, box).

The full feature maps are shipped to device DRAM but only the ~440KB/core the
loss actually references is ever read by the kernel, so HW time sits far under
the streaming-memory roofline.

All SBUF tensors participating in ops against strided channel slices (cls
[64:144] / dist [0:64]) are allocated [128, 6, 144] and sliced identically, so
every instruction's operands lower to the same access-pattern shape.
"""

import sys
from contextlib import ExitStack

import numpy as np

for _p in ("/opt/trn_rl_repo", "/root/.axon_site/_ro/trn_rl_repo"):
    if _p not in sys.path:
        sys.path.append(_p)

N_CLASSES = 80
N_BINS = 16
ND = 4 * N_BINS             # 64 dist channels
B, T = 32, 64
M = 8                       # cores
BL = B // M                 # images per core
C = N_CLASSES + ND          # 144
HWS = [(80, 80), (40, 40), (20, 20)]
ROWS = 3 * BL * T           # 768 gathered rows per core
NBLK = ROWS // 128          # 6
ROWS_PER_LAYER = BL * T     # 256
N_FT = BL * sum(h * w for h, w in HWS)  # 33600 rows in the concat feature table
LAYER_BASE = [0, BL * 6400, BL * 6400 + BL * 1600]

_PROG = None


def _build_program(repeat=1):
    import concourse.bass as bass
    import concourse.tile as tile
    from concourse import bacc, mybir

    f32 = mybir.dt.float32
    i32 = mybir.dt.int32
    Act = mybir.ActivationFunctionType
    Alu = mybir.AluOpType
    AxX = mybir.AxisListType.X
    AxXY = mybir.AxisListType.XY

    nc = bacc.Bacc("TRN2", debug=False, num_devices=M)

    ft_d = nc.dram_tensor("ft", [N_FT, C], f32, kind="ExternalInput").ap()
    idx_d = nc.dram_tensor("idx", [128, NBLK], i32, kind="ExternalInput").ap()
    aux_d = nc.dram_tensor("aux", [128, NBLK, C], f32, kind="ExternalInput").ap()
    msk_d = nc.dram_tensor("msk", [128, NBLK], f32, kind="ExternalInput").ap()
    ws_d = nc.dram_tensor("ws", [128, NBLK, 4], f32, kind="ExternalInput").ap()
    out_d = nc.dram_tensor("out", [1, 2], f32, kind="ExternalOutput").ap()

    with tile.TileContext(nc) as tc, ExitStack() as ctx:
        sb = ctx.enter_context(tc.tile_pool(name="sb", bufs=1))
        ps = ctx.enter_context(tc.tile_pool(name="ps", bufs=1, space="PSUM"))

        for _ in range(repeat):
            idx = sb.tile([128, NBLK], i32)
            aux = sb.tile([128, NBLK, C], f32)
            msk = sb.tile([128, NBLK], f32)
            ws = sb.tile([128, NBLK, 4], f32)
            nc.sync.dma_start(out=idx[:], in_=idx_d)
            nc.sync.dma_start(out=aux[:], in_=aux_d)
            nc.sync.dma_start(out=msk[:], in_=msk_d)
            nc.sync.dma_start(out=ws[:], in_=ws_d)

            # Indirect gathers: the HW descriptor engine consumes one index
            # per partition per DMA, so one call per 128-row block.
            G = sb.tile([128, NBLK, C], f32)
            for blk in range(NBLK):
                nc.gpsimd.indirect_dma_start(
                    out=G[:, blk, :],
                    out_offset=None,
                    in_=ft_d,
                    in_offset=bass.IndirectOffsetOnAxis(
                        ap=idx[:, blk : blk + 1], axis=0
                    ),
                )

            X = G[:, :, ND:]      # [128, 6, 80] class logits
            D = G[:, :, :ND]      # [128, 6, 64] dist logits
            OH = aux[:, :, ND:]   # one-hot(tgt_cls)
            WD = aux[:, :, :ND]   # DFL lo/hi bin weights

            EB = sb.tile([128, NBLK, C], f32)   # exp(G)
            TB = sb.tile([128, NBLK, C], f32)   # G * aux
            S = sb.tile([128, NBLK], f32)
            L = sb.tile([128, NBLK], f32)
            XS = sb.tile([128, NBLK], f32)
            CE = sb.tile([128, NBLK], f32)
            PT = sb.tile([128, NBLK], f32)
            Q2 = sb.tile([128, NBLK], f32)
            F = sb.tile([128, NBLK], f32)
            FM = sb.tile([128, NBLK], f32)
            P2 = sb.tile([128, 2], f32)

            # ---- focal classification loss ----
            nc.scalar.activation(out=EB[:, :, ND:], in_=X, func=Act.Exp)
            nc.vector.tensor_reduce(out=S[:], in_=EB[:, :, ND:], axis=AxX, op=Alu.add)
            nc.scalar.activation(out=L[:], in_=S[:], func=Act.Ln)
            nc.vector.tensor_tensor(out=TB[:, :, ND:], in0=X, in1=OH, op=Alu.mult)
            nc.vector.tensor_reduce(out=XS[:], in_=TB[:, :, ND:], axis=AxX, op=Alu.add)
            nc.vector.tensor_tensor(out=CE[:], in0=L[:], in1=XS[:], op=Alu.subtract)
            nc.scalar.activation(out=PT[:], in_=CE[:], func=Act.Exp, scale=-1.0)
            nc.scalar.activation(
                out=Q2[:], in_=PT[:], func=Act.Square, scale=-1.0, bias=1.0
            )
            nc.vector.tensor_tensor(out=F[:], in0=Q2[:], in1=CE[:], op=Alu.mult)
            nc.vector.tensor_tensor(out=FM[:], in0=F[:], in1=msk[:], op=Alu.mult)
            nc.vector.tensor_reduce(out=P2[:, 0:1], in_=FM[:], axis=AxX, op=Alu.add)

            # ---- DFL box loss ----
            # dl = -(lps[lo]*wl + lps[hi]*wr), lps = D - log(sum(exp(D))) per
            # 16-bin group  =>  box = sum(ws * LD) - sum(WD * D) with
            # host-baked sparse weights (ws[.,s] = wl+wr on the selected
            # row/side, WD holds wl/wr at the lo/hi bins of selected rows).
            SD = sb.tile([128, NBLK, 4], f32)
            LD = sb.tile([128, NBLK, 4], f32)
            T1 = sb.tile([128, NBLK, 4], f32)
            Acc1 = sb.tile([128, 1], f32)
            Acc2 = sb.tile([128, 1], f32)

            nc.scalar.activation(out=EB[:, :, :ND], in_=D, func=Act.Exp)
            nc.vector.tensor_reduce(
                out=SD[:],
                in_=EB[:, :, :ND].rearrange("p r (s n) -> p r s n", n=N_BINS),
                axis=AxX,
                op=Alu.add,
                opt_output=False,
            )
            nc.scalar.activation(out=LD[:], in_=SD[:], func=Act.Ln)
            nc.vector.tensor_tensor(out=T1[:], in0=LD[:], in1=ws[:], op=Alu.mult)
            nc.vector.tensor_reduce(out=Acc1[:], in_=T1[:], axis=AxXY, op=Alu.add)
            nc.vector.tensor_tensor(out=TB[:, :, :ND], in0=D, in1=WD, op=Alu.mult)
            nc.vector.tensor_reduce(
                out=Acc2[:], in_=TB[:, :, :ND], axis=AxXY, op=Alu.add
            )
            nc.vector.tensor_tensor(
                out=P2[:, 1:2], in0=Acc1[:], in1=Acc2[:], op=Alu.subtract
            )

            # ---- partition sum via PE: ones[128,1].T @ P2[128,2] -> [1,2] ----
            ONES = sb.tile([128, 1], f32)
            nc.vector.memset(ONES[:], 1.0)
            PS = ps.tile([1, 2], f32)
            nc.tensor.matmul(out=PS[:], lhsT=ONES[:], rhs=P2[:], start=True, stop=True)
            O = sb.tile([1, 2], f32)
            nc.vector.tensor_copy(out=O[:], in_=PS[:])
            nc.sync.dma_start(out=out_d, in_=O[:])

    nc.compile()
    return nc


def _host_prep(feat0, feat1, feat2, tgt_box, tgt_cls, tgt_layer):
    """Build the 8 per-core input maps."""
    f32 = np.float32
    feats = [feat0, feat1, feat2]
    cx, cy = tgt_box[..., 0], tgt_box[..., 1]
    wv, hv = tgt_box[..., 2], tgt_box[..., 3]

    # Per-layer integer grid positions (bit-exact with the f32 reference math).
    fx, fy = {}, {}
    for li, (H, W) in enumerate(HWS):
        fx[li] = np.clip((cx * f32(W)).astype(np.int32), 0, W - 1)  # [B,T]
        fy[li] = np.clip((cy * f32(H)).astype(np.int32), 0, H - 1)

    # Per-layer DFL quantities (the reference's "last matching target" bug).
    tidx = np.arange(T)
    bv = np.arange(B)
    dfl = {}
    for li, (H, W) in enumerate(HWS):
        mask_l = tgt_layer == li
        last = np.max(np.where(mask_l, tidx[None, :], -1), axis=1)  # [B]
        has = last >= 0
        last_c = np.maximum(last, 0)
        lw = np.maximum(wv[bv, last_c], f32(0.0)) * f32(0.5)
        lh = np.maximum(hv[bv, last_c], f32(0.0)) * f32(0.5)
        gt = np.stack([lw * f32(W), lh * f32(H), lw * f32(W), lh * f32(H)], axis=1)
        tq = np.clip(gt, f32(0.0), f32(N_BINS - 1 - 1e-6))
        lo = np.floor(tq)
        wl = (lo + f32(1.0)) - tq
        wr = tq - lo
        lo_i = lo.astype(np.int32)
        hi_i = np.minimum(lo_i + 1, N_BINS - 1)
        dfl[li] = (last_c, has, wl, wr, lo_i, hi_i)

    blv = np.arange(BL)
    in_maps = []
    for m in range(M):
        b0 = m * BL
        sl = slice(b0, b0 + BL)
        im = {}

        ft = np.empty((N_FT, C), f32)
        for li, (H, W) in enumerate(HWS):
            base = LAYER_BASE[li]
            src = feats[li][sl].reshape(BL, C, H * W).transpose(0, 2, 1)
            ft[base : base + BL * H * W] = src.reshape(BL * H * W, C)
        im["ft"] = ft

        idx = np.zeros((128, NBLK), np.int32)
        aux = np.zeros((128, NBLK, C), f32)
        msk = np.zeros((128, NBLK), f32)
        ws = np.zeros((128, NBLK, 4), f32)

        rr_bt = blv[:, None] * T + tidx[None, :]  # [BL, T] row-within-layer
        for li, (H, W) in enumerate(HWS):
            rr = li * ROWS_PER_LAYER + rr_bt
            p, blk = rr % 128, rr // 128
            idx[p, blk] = (
                LAYER_BASE[li] + blv[:, None] * (H * W) + fy[li][sl] * W + fx[li][sl]
            ).astype(np.int32)
            aux[p, blk, ND + tgt_cls[sl]] = f32(1.0)
            msk[p, blk] = (tgt_layer[sl] == li).astype(f32)

            last_c, has, wl, wr, lo_i, hi_i = dfl[li]
            for bl in range(BL):
                b = b0 + bl
                if not has[b]:
                    continue
                r = li * ROWS_PER_LAYER + bl * T + last_c[b]
                p1, blk1 = r % 128, r // 128
                for s in range(4):
                    aux[p1, blk1, s * N_BINS + lo_i[b, s]] = wl[b, s]
                    aux[p1, blk1, s * N_BINS + hi_i[b, s]] = wr[b, s]
                    ws[p1, blk1, s] = wl[b, s] + wr[b, s]

        im["idx"] = idx
        im["aux"] = aux
        im["msk"] = msk
        im["ws"] = ws
        in_maps.append(im)
    return in_maps


def kernel(feat0, feat1, feat2, tgt_box, tgt_cls, tgt_layer):
    global _PROG
    from concourse.bass_utils import run_bass_kernel_spmd

    in_maps = _host_prep(feat0, feat1, feat2, tgt_box, tgt_cls, tgt_layer)
    if _PROG is None:
        _PROG = _build_program()
    res = run_bass_kernel_spmd(_PROG, in_maps, list(range(M))).results
    parts = np.stack([res[i]["out"][0] for i in range(M)])  # [M, 2]
    cls_tot = parts[:, 0].sum(dtype=np.float32)
    box_tot = parts[:, 1].sum(dtype=np.float32)
    total = np.float32(cls_tot + box_tot)
    return (total, np.float32(cls_tot), np.float32(box_tot))


# revision 14
# speedup vs baseline: 628.9803x; 1.8043x over previous
"""DetectionLoss Trainium2 kernel.

Strategy (data-parallel over batch, per sharding hint):
- Shard B=32 across 8 cores (4 images each).
- Host-side prep per core: transpose feature shards to channel-last and
  concatenate all 3 pyramid levels into one [33600, 144] DRAM tensor, so each
  target's 144 channel values are one contiguous row; precompute gather row
  indices and one-hot / DFL weight tensors from the (tiny) target tensors.
- Key algebraic reduction: the reference computes focal loss for every
  (target, layer) pair but masks all pairs where tgt_layer != layer, and its
  DFL rows are positions of targets whose own layer matches. So only each
  target's row AT ITS OWN LAYER can contribute: 4 img * 64 tgt = 256 rows per
  core, fetched by two 128-row indirect DMAs (the HW descriptor engine takes
  one index per partition per DMA).
- Device: gather the 256 rows, compute focal cls loss and DFL box loss
  on-chip (exp/ln on Act engine, elementwise/reduces on DVE), emitting
  per-partition partials [128, (cls, box)].
- Host: sum partials over partitions and cores -> (total, cls, box).

The full feature maps are shipped to device DRAM but only ~150KB/core is ever
read by the kernel, so HW time sits far under the streaming-memory roofline.

SBUF tensors that appear in ops against strided channel slices (cls [64:144] /
dist [0:64]) are allocated [128, 2, 144] and sliced identically so every
instruction's operands lower to the same access-pattern shape (a bass_interp
requirement; hardware is indifferent).
"""

import sys
from contextlib import ExitStack

import numpy as np

for _p in ("/opt/trn_rl_repo", "/root/.axon_site/_ro/trn_rl_repo"):
    if _p not in sys.path:
        sys.path.append(_p)

N_CLASSES = 80
N_BINS = 16
ND = 4 * N_BINS             # 64 dist channels
B, T = 32, 64
M = 8                       # cores
BL = B // M                 # images per core
C = N_CLASSES + ND          # 144
HWS = [(80, 80), (40, 40), (20, 20)]
ROWS = BL * T               # 256 gathered rows per core
NBLK = ROWS // 128          # 2
N_FT = BL * sum(h * w for h, w in HWS)  # 33600 rows in the concat feature table
LAYER_BASE = [0, BL * 6400, BL * 6400 + BL * 1600]

_PROG = None


def _build_program(repeat=1):
    import concourse.bass as bass
    import concourse.tile as tile
    from concourse import bacc, mybir

    f32 = mybir.dt.float32
    i32 = mybir.dt.int32
    Act = mybir.ActivationFunctionType
    Alu = mybir.AluOpType
    AxX = mybir.AxisListType.X
    AxXY = mybir.AxisListType.XY

    nc = bacc.Bacc("TRN2", debug=False, num_devices=M)

    ft_d = nc.dram_tensor("ft", [N_FT, C], f32, kind="ExternalInput").ap()
    idx_d = nc.dram_tensor("idx", [128, NBLK], i32, kind="ExternalInput").ap()
    aux_d = nc.dram_tensor("aux", [128, NBLK, C], f32, kind="ExternalInput").ap()
    ws_d = nc.dram_tensor("ws", [128, NBLK, 4], f32, kind="ExternalInput").ap()
    out_d = nc.dram_tensor("out", [128, 2], f32, kind="ExternalOutput").ap()

    with tile.TileContext(nc) as tc, ExitStack() as ctx:
        sb = ctx.enter_context(tc.tile_pool(name="sb", bufs=1))

        for _ in range(repeat):
            idx = sb.tile([128, NBLK], i32)
            aux = sb.tile([128, NBLK, C], f32)
            ws = sb.tile([128, NBLK, 4], f32)
            # idx gates the gathers -> its own (sync) queue; aux/ws aren't
            # needed until several ops in, so keep them off that queue.
            nc.sync.dma_start(out=idx[:], in_=idx_d)
            nc.scalar.dma_start(out=aux[:], in_=aux_d)
            nc.sync.dma_start(out=ws[:], in_=ws_d)

            # Indirect gathers: one 128-index DMA per block.
            G = sb.tile([128, NBLK, C], f32)
            for blk in range(NBLK):
                nc.gpsimd.indirect_dma_start(
                    out=G[:, blk, :],
                    out_offset=None,
                    in_=ft_d,
                    in_offset=bass.IndirectOffsetOnAxis(
                        ap=idx[:, blk : blk + 1], axis=0
                    ),
                )

            X = G[:, :, ND:]      # [128, 2, 80] class logits
            D = G[:, :, :ND]      # [128, 2, 64] dist logits
            OH = aux[:, :, ND:]   # one-hot(tgt_cls)
            WD = aux[:, :, :ND]   # DFL lo/hi bin weights

            EB = sb.tile([128, NBLK, C], f32)   # exp(G)
            TB = sb.tile([128, NBLK, C], f32)   # G * aux
            S = sb.tile([128, NBLK], f32)
            L = sb.tile([128, NBLK], f32)
            XS = sb.tile([128, NBLK], f32)
            CE = sb.tile([128, NBLK], f32)
            PT = sb.tile([128, NBLK], f32)
            Q2 = sb.tile([128, NBLK], f32)
            F = sb.tile([128, NBLK], f32)
            P2 = sb.tile([128, 2], f32)

            # ---- focal classification loss (every row contributes) ----
            # One exp over all 144 channels serves both the cls softmax (last
            # 80) and the DFL softmax (first 64); same for the aux multiply.
            nc.scalar.activation(out=EB[:], in_=G[:], func=Act.Exp)
            nc.vector.tensor_reduce(out=S[:], in_=EB[:, :, ND:], axis=AxX, op=Alu.add)
            nc.scalar.activation(out=L[:], in_=S[:], func=Act.Ln)
            nc.vector.tensor_tensor(out=TB[:], in0=G[:], in1=aux[:], op=Alu.mult)
            nc.vector.tensor_reduce(out=XS[:], in_=TB[:, :, ND:], axis=AxX, op=Alu.add)
            nc.vector.tensor_tensor(out=CE[:], in0=L[:], in1=XS[:], op=Alu.subtract)
            nc.scalar.activation(out=PT[:], in_=CE[:], func=Act.Exp, scale=-1.0)
            nc.scalar.activation(
                out=Q2[:], in_=PT[:], func=Act.Square, scale=-1.0, bias=1.0
            )
            nc.vector.tensor_tensor(out=F[:], in0=Q2[:], in1=CE[:], op=Alu.mult)
            nc.vector.tensor_reduce(out=P2[:, 0:1], in_=F[:], axis=AxX, op=Alu.add)

            # ---- DFL box loss ----
            # dl = -(lps[lo]*wl + lps[hi]*wr), lps = D - log(sum(exp(D))) per
            # 16-bin group  =>  box = sum(ws * LD) - sum(WD * D) with
            # host-baked sparse weights (ws[.,s] = wl+wr on the selected
            # row/side, WD holds wl/wr at the lo/hi bins of selected rows).
            SD = sb.tile([128, NBLK, 4], f32)
            LD = sb.tile([128, NBLK, 4], f32)
            T1 = sb.tile([128, NBLK, 4], f32)
            Acc1 = sb.tile([128, 1], f32)
            Acc2 = sb.tile([128, 1], f32)

            nc.vector.tensor_reduce(
                out=SD[:],
                in_=EB[:, :, :ND].rearrange("p r (s n) -> p r s n", n=N_BINS),
                axis=AxX,
                op=Alu.add,
                opt_output=False,
            )
            nc.scalar.activation(out=LD[:], in_=SD[:], func=Act.Ln)
            nc.vector.tensor_tensor(out=T1[:], in0=LD[:], in1=ws[:], op=Alu.mult)
            nc.vector.tensor_reduce(out=Acc1[:], in_=T1[:], axis=AxXY, op=Alu.add)
            nc.vector.tensor_reduce(
                out=Acc2[:], in_=TB[:, :, :ND], axis=AxXY, op=Alu.add
            )
            nc.vector.tensor_tensor(
                out=P2[:, 1:2], in0=Acc1[:], in1=Acc2[:], op=Alu.subtract
            )

            nc.sync.dma_start(out=out_d, in_=P2[:])

    nc.compile()
    return nc


def _host_prep(feat0, feat1, feat2, tgt_box, tgt_cls, tgt_layer):
    """Build the 8 per-core input maps."""
    f32 = np.float32
    feats = [feat0, feat1, feat2]
    cx, cy = tgt_box[..., 0], tgt_box[..., 1]
    wv, hv = tgt_box[..., 2], tgt_box[..., 3]

    # Per-layer integer grid positions (bit-exact with the f32 reference math).
    fx = np.empty((3, B, T), np.int64)
    fy = np.empty((3, B, T), np.int64)
    for li, (H, W) in enumerate(HWS):
        fx[li] = np.clip((cx * f32(W)).astype(np.int32), 0, W - 1)
        fy[li] = np.clip((cy * f32(H)).astype(np.int32), 0, H - 1)

    # Each target's row index in the concat table, at its own layer.
    lb = np.array(LAYER_BASE, np.int64)[tgt_layer]            # [B,T]
    hw_l = np.array([h * w for h, w in HWS], np.int64)[tgt_layer]
    w_l = np.array([w for _, w in HWS], np.int64)[tgt_layer]
    li_idx = tgt_layer[None]                                   # [1,B,T]
    fx_own = np.take_along_axis(fx, li_idx, 0)[0]              # [B,T]
    fy_own = np.take_along_axis(fy, li_idx, 0)[0]
    bl_all = (np.arange(B) % BL)[:, None]
    row_own = lb + bl_all * hw_l + fy_own * w_l + fx_own       # [B,T]

    # Per-layer DFL quantities (the reference's "last matching target" bug).
    tidx = np.arange(T)
    bv = np.arange(B)
    dfl = {}
    for li, (H, W) in enumerate(HWS):
        mask_l = tgt_layer == li
        last = np.max(np.where(mask_l, tidx[None, :], -1), axis=1)  # [B]
        has = last >= 0
        last_c = np.maximum(last, 0)
        lw = np.maximum(wv[bv, last_c], f32(0.0)) * f32(0.5)
        lh = np.maximum(hv[bv, last_c], f32(0.0)) * f32(0.5)
        gt = np.stack([lw * f32(W), lh * f32(H), lw * f32(W), lh * f32(H)], axis=1)
        tq = np.clip(gt, f32(0.0), f32(N_BINS - 1 - 1e-6))
        lo = np.floor(tq)
        wl = (lo + f32(1.0)) - tq
        wr = tq - lo
        lo_i = lo.astype(np.int32)
        hi_i = np.minimum(lo_i + 1, N_BINS - 1)
        dfl[li] = (last_c, has, wl, wr, lo_i, hi_i)

    blv = np.arange(BL)
    rr = blv[:, None] * T + tidx[None, :]          # [BL,T] row id within core
    p_all, blk_all = rr % 128, rr // 128

    in_maps = []
    for m in range(M):
        b0 = m * BL
        sl = slice(b0, b0 + BL)
        im = {}

        ft = np.empty((N_FT, C), f32)
        for li, (H, W) in enumerate(HWS):
            base = LAYER_BASE[li]
            src = feats[li][sl].reshape(BL, C, H * W).transpose(0, 2, 1)
            ft[base : base + BL * H * W] = src.reshape(BL * H * W, C)
        im["ft"] = ft

        idx = np.zeros((128, NBLK), np.int32)
        aux = np.zeros((128, NBLK, C), f32)
        ws = np.zeros((128, NBLK, 4), f32)

        idx[p_all, blk_all] = row_own[sl].astype(np.int32)
        aux[p_all, blk_all, ND + tgt_cls[sl]] = f32(1.0)

        for li in range(3):
            last_c, has, wl, wr, lo_i, hi_i = dfl[li]
            for bl in range(BL):
                b = b0 + bl
                if not has[b]:
                    continue
                r = bl * T + last_c[b]
                p1, blk1 = r % 128, r // 128
                for s in range(4):
                    aux[p1, blk1, s * N_BINS + lo_i[b, s]] = wl[b, s]
                    aux[p1, blk1, s * N_BINS + hi_i[b, s]] = wr[b, s]
                    ws[p1, blk1, s] = wl[b, s] + wr[b, s]

        im["idx"] = idx
        im["aux"] = aux
        im["ws"] = ws
        in_maps.append(im)
    return in_maps


def kernel(feat0, feat1, feat2, tgt_box, tgt_cls, tgt_layer):
    global _PROG
    from concourse.bass_utils import run_bass_kernel_spmd

    in_maps = _host_prep(feat0, feat1, feat2, tgt_box, tgt_cls, tgt_layer)
    if _PROG is None:
        _PROG = _build_program()
    res = run_bass_kernel_spmd(_PROG, in_maps, list(range(M))).results
    parts = np.stack([res[i]["out"] for i in range(M)])  # [M, 128, 2]
    cls_tot = parts[:, :, 0].sum(dtype=np.float32)
    box_tot = parts[:, :, 1].sum(dtype=np.float32)
    total = np.float32(cls_tot + box_tot)
    return (total, np.float32(cls_tot), np.float32(box_tot))
